# revision 8
# baseline (speedup 1.0000x reference)
"""Trainium2 Bass kernel for CDUserItemAttention.

Math (per reference):
    q_u = Q_user @ W_q_user + b_q_user   (same for k_u, v_u, q_i, k_i)
    attn = 0.9*softmax(q_u k_u^T / sqrt(512)) + 0.1*softmax(q_i k_i^T / sqrt(512))
    out  = attn @ v_u
    returns (out [4096,512], attn [4096,4096])

Sharding: rows of Q_user/Q_item (512 queries per core) across 8 cores;
K/V/weights replicated.  Softmax rows are core-local, so no collectives.

Per-core kernel structure (matmuls in fp16, everything else fp32):
  - host pre-transposes/packs activations so every projection contracts over
    the partition dim with no on-device transposes:
      q^T/k^T projections: out[dout,n] = W-chunk.T @ X^T-chunk, PSUM-accum
      v  projection:       out[k,dout] = vT-chunk.T @ W_v  (+ ones-row bias)
  - scores S[q,k] accumulate over 4 d-chunks in PSUM; ScalarE evicts with a
    fused exp(x/sqrt(512)) + per-row accumulation (softmax skips the
    max-subtraction: logits are in [-2, 2] for this problem's distribution)
  - blend on VectorE: attn = (0.9/l_u)*E_u + (0.1/l_i)*E_i, in place
  - attn tiles are PE-transposed (scaled by 64 to stay in fp16 normal range)
    for the attn @ v_u matmul; the output eviction divides by 64.
"""

import os
import sys

os.environ.setdefault("MYCRO_LOCAL_CACHE", "1")
if "/opt/trn_rl_repo" not in sys.path:
    sys.path.insert(0, "/opt/trn_rl_repo")

from contextlib import ExitStack

import numpy as np

import concourse.bacc as bacc
import concourse.bass as bass
import concourse.tile as tile
from concourse import mybir
from concourse.bass_utils import run_bass_kernel_spmd

N = 4096
D = 512
NCORES = 8
P = 128
DC = D // P          # 4 contraction chunks of 128
NB = 512             # key-block width (matmul moving dim)
F32 = mybir.dt.float32
ALPHA = 0.9
SCALE = 1.0 / float(np.sqrt(np.float32(D)))
AT_SCALE = 64.0      # keeps attn^T fp16 tiles in normal range

AF = mybir.ActivationFunctionType
ALU = mybir.AluOpType
AX = mybir.AxisListType

MM_DT = mybir.dt.float16


def build_program(nq=N // NCORES, nk=N, mm_dt=MM_DT):
    """Build the per-core SPMD Bass program.

    nq: queries handled by this core; nk: total keys. Parameterized so a
    shrunken version can run under CoreSim quickly.
    """
    nt = nq // P          # q tiles
    nj = nk // NB         # key blocks (of 512)
    nm = nk // P          # key P-tiles
    MD = mm_dt
    at_scale = AT_SCALE if mm_dt == mybir.dt.float16 else 1.0

    nc = bacc.Bacc("TRN2", debug=False)

    # ---- I/O (host packs X^T chunk-contiguous: [P, DC, n] flattened) ----
    quT = nc.dram_tensor("quT", [P, DC * nq], MD, kind="ExternalInput")
    qiT = nc.dram_tensor("qiT", [P, DC * nq], MD, kind="ExternalInput")
    kuT = nc.dram_tensor("kuT", [P, DC * nk], MD, kind="ExternalInput")
    kiT = nc.dram_tensor("kiT", [P, DC * nk], MD, kind="ExternalInput")
    vTp = nc.dram_tensor("vTp", [P, DC * nk], MD, kind="ExternalInput")
    Wqu = nc.dram_tensor("Wqu", [P, DC * D], MD, kind="ExternalInput")
    Wku = nc.dram_tensor("Wku", [P, DC * D], MD, kind="ExternalInput")
    Wqi = nc.dram_tensor("Wqi", [P, DC * D], MD, kind="ExternalInput")
    Wki = nc.dram_tensor("Wki", [P, DC * D], MD, kind="ExternalInput")
    Wv = nc.dram_tensor("Wv", [P, DC * D], MD, kind="ExternalInput")
    bv = nc.dram_tensor("bv", [1, D], MD, kind="ExternalInput")
    bqu = nc.dram_tensor("bqu", [P, DC], F32, kind="ExternalInput")
    bku = nc.dram_tensor("bku", [P, DC], F32, kind="ExternalInput")
    bqi = nc.dram_tensor("bqi", [P, DC], F32, kind="ExternalInput")
    bki = nc.dram_tensor("bki", [P, DC], F32, kind="ExternalInput")
    ident = nc.dram_tensor("ident", [P, P], F32, kind="ExternalInput")
    attn_c = nc.dram_tensor("attn_c", [nq, nk], F32, kind="ExternalOutput")
    out_c = nc.dram_tensor("out_c", [nq, D], F32, kind="ExternalOutput")

    c3 = lambda d_, n_: d_.rearrange("p (c x) -> p c x", c=DC)

    with tile.TileContext(nc) as tc:
        with ExitStack() as ctx:
            const = ctx.enter_context(tc.tile_pool(name="const", bufs=1))
            ident_sb = const.tile([P, P], F32, tag="ident")
            nc.sync.dma_start(out=ident_sb, in_=ident[:, :])

            # per-row softmax partial sums / scalars
            lp = ctx.enter_context(tc.tile_pool(name="lp", bufs=1))
            lpart_u = lp.tile([P, nt * nj], F32, tag="lpu")
            lpart_i = lp.tile([P, nt * nj], F32, tag="lpi")
            lsum = lp.tile([P, 2 * nt], F32, tag="lsum")      # [u(0..nt) | i(nt..)]
            linv = lp.tile([P, 2 * nt], F32, tag="linv")
            ab = lp.tile([P, 2 * nt], F32, tag="ab")          # a_u(t), b_i(t)

            ei_pool = ctx.enter_context(tc.tile_pool(name="ei", bufs=nt))
            ei = [ei_pool.tile([P, nk], F32, tag="ei", name=f"ei{t}")
                  for t in range(nt)]

            def stream_phase(qT_d, kT_d, Wq_d, Wk_d, bq_d, bk_d, e_tiles, lpart):
                """Project q^T and (streamed) k^T; exp(S/sqrt(d)) into e_tiles
                with per-row partial sums into lpart."""
                with ExitStack() as pctx:
                    wpool = pctx.enter_context(tc.tile_pool(name="w", bufs=1))
                    w_q = wpool.tile([P, DC, D], MD, tag="wq")
                    w_k = wpool.tile([P, DC, D], MD, tag="wk")
                    b_q = wpool.tile([P, DC], F32, tag="bq")
                    b_k = wpool.tile([P, DC], F32, tag="bk")
                    for c in range(DC):
                        nc.sync.dma_start(out=w_q[:, c, :],
                                          in_=Wq_d[:, D * c:D * (c + 1)])
                        nc.sync.dma_start(out=w_k[:, c, :],
                                          in_=Wk_d[:, D * c:D * (c + 1)])
                    nc.sync.dma_start(out=b_q, in_=bq_d[:, :])
                    nc.sync.dma_start(out=b_k, in_=bk_d[:, :])

                    qin_pool = pctx.enter_context(tc.tile_pool(name="qin", bufs=1))
                    qp_pool = pctx.enter_context(tc.tile_pool(name="qp", bufs=DC))
                    psum = pctx.enter_context(
                        tc.tile_pool(name="mmps", bufs=4, space="PSUM"))

                    # q^T projection: qP[c] rows 128c.. of q^T  [128 dout, nq]
                    q_in = qin_pool.tile([P, DC, nq], MD, tag="qin")
                    for c in range(DC):
                        nc.sync.dma_start(out=q_in[:, c, :],
                                          in_=qT_d[:, nq * c:nq * (c + 1)])
                    qP = []
                    for tau in range(DC):
                        ps = psum.tile([P, nq], F32, tag="ps")
                        for c in range(DC):
                            nc.tensor.matmul(
                                ps, w_q[:, c, P * tau:P * (tau + 1)], q_in[:, c, :],
                                start=(c == 0), stop=(c == DC - 1))
                        o = qp_pool.tile([P, nq], MD, tag="qp")
                        nc.scalar.activation(o, ps, AF.Identity,
                                             bias=b_q[:, tau:tau + 1])
                        qP.append(o)

                    kin_pool = pctx.enter_context(tc.tile_pool(name="kin", bufs=2))
                    kp_pool = pctx.enter_context(tc.tile_pool(name="kp", bufs=2 * DC))
                    for j in range(nj):
                        k_in = kin_pool.tile([P, DC, NB], MD, tag="kin")
                        for c in range(DC):
                            nc.sync.dma_start(
                                out=k_in[:, c, :],
                                in_=kT_d[:, nk * c + NB * j:nk * c + NB * (j + 1)])
                        kP = []
                        for tau in range(DC):
                            ps = psum.tile([P, NB], F32, tag="ps")
                            for c in range(DC):
                                nc.tensor.matmul(
                                    ps, w_k[:, c, P * tau:P * (tau + 1)],
                                    k_in[:, c, :],
                                    start=(c == 0), stop=(c == DC - 1))
                            o = kp_pool.tile([P, NB], MD, tag="kp")
                            nc.scalar.activation(o, ps, AF.Identity,
                                                 bias=b_k[:, tau:tau + 1])
                            kP.append(o)
                        for t in range(nt):
                            ps = psum.tile([P, NB], F32, tag="ps")
                            for c in range(DC):
                                nc.tensor.matmul(
                                    ps, qP[c][:, P * t:P * (t + 1)], kP[c],
                                    start=(c == 0), stop=(c == DC - 1))
                            nc.scalar.activation(
                                e_tiles[t][:, NB * j:NB * (j + 1)], ps, AF.Exp,
                                scale=SCALE,
                                accum_out=lpart[:, t * nj + j:t * nj + j + 1])

            with ExitStack() as uctx:
                eu_pool = uctx.enter_context(tc.tile_pool(name="eu", bufs=nt))
                eu = [eu_pool.tile([P, nk], F32, tag="eu", name=f"eu{t}")
                      for t in range(nt)]
                stream_phase(quT, kuT, Wqu, Wku, bqu, bku, eu, lpart_u)
                stream_phase(qiT, kiT, Wqi, Wki, bqi, bki, ei, lpart_i)

                # ---- softmax scalars + blend ----
                for t in range(nt):
                    nc.vector.reduce_sum(lsum[:, t:t + 1],
                                         lpart_u[:, t * nj:(t + 1) * nj],
                                         axis=AX.X, op=ALU.add)
                    nc.vector.reduce_sum(lsum[:, nt + t:nt + t + 1],
                                         lpart_i[:, t * nj:(t + 1) * nj],
                                         axis=AX.X, op=ALU.add)
                for t in range(nt):
                    nc.vector.reciprocal(linv[:, t:t + 1], lsum[:, t:t + 1])
                    nc.vector.reciprocal(linv[:, nt + t:nt + t + 1],
                                         lsum[:, nt + t:nt + t + 1])
                    nc.vector.tensor_scalar_mul(ab[:, t:t + 1],
                                                linv[:, t:t + 1], ALPHA)
                    nc.vector.tensor_scalar_mul(ab[:, nt + t:nt + t + 1],
                                                linv[:, nt + t:nt + t + 1],
                                                1.0 - ALPHA)
                for t in range(nt):
                    # ei[t] = eu[t]*a_u + ei[t]*b_i  (blended attn rows)
                    nc.vector.tensor_scalar_mul(ei[t], ei[t],
                                                ab[:, nt + t:nt + t + 1])
                    nc.vector.scalar_tensor_tensor(
                        ei[t], eu[t], ab[:, t:t + 1], ei[t],
                        op0=ALU.mult, op1=ALU.add)
                    nc.sync.dma_start(out=attn_c[P * t:P * (t + 1), :], in_=ei[t])

            # ---- v projection (streamed) + attn @ v ----
            with ExitStack() as vctx:
                wvp = vctx.enter_context(tc.tile_pool(name="wv", bufs=1))
                wv = wvp.tile([P, DC, D], MD, tag="wv")
                wv1 = wvp.tile([1, D], MD, tag="wvbias")
                ones1 = wvp.tile([1, P], MD, tag="ones1")
                for c in range(DC):
                    nc.sync.dma_start(out=wv[:, c, :],
                                      in_=Wv[:, D * c:D * (c + 1)])
                nc.sync.dma_start(out=wv1, in_=bv[:, :])
                nc.vector.memset(ones1, 1.0)

                vin_pool = vctx.enter_context(tc.tile_pool(name="vin", bufs=3))
                vt_pool = vctx.enter_context(tc.tile_pool(name="vt", bufs=3))
                at_pool = vctx.enter_context(tc.tile_pool(name="at", bufs=6))
                osb_pool = vctx.enter_context(tc.tile_pool(name="osb", bufs=nt))
                vps = vctx.enter_context(
                    tc.tile_pool(name="vps", bufs=2, space="PSUM"))
                trps = vctx.enter_context(
                    tc.tile_pool(name="trps", bufs=2, space="PSUM"))
                outps = vctx.enter_context(
                    tc.tile_pool(name="outps", bufs=nt, space="PSUM"))

                ops = [outps.tile([P, D], F32, tag="ops", name=f"ops{t}")
                       for t in range(nt)]
                for m in range(nm):
                    v_in = vin_pool.tile([P, DC, P], MD, tag="vin")
                    for c in range(DC):
                        nc.sync.dma_start(
                            out=v_in[:, c, :],
                            in_=vTp[:, nk * c + P * m:nk * c + P * (m + 1)])

                    vp = vps.tile([P, D], F32, tag="vps")
                    for c in range(DC):
                        nc.tensor.matmul(vp, v_in[:, c, :], wv[:, c, :],
                                         start=(c == 0), stop=False)
                    nc.tensor.matmul(vp, ones1, wv1, start=False, stop=True)
                    v_m = vt_pool.tile([P, D], MD, tag="vt")
                    nc.scalar.copy(v_m, vp)

                    for t in range(nt):
                        tp = trps.tile([P, P], F32, tag="trps")
                        nc.tensor.transpose(tp, ei[t][:, P * m:P * (m + 1)],
                                            ident_sb)
                        at = at_pool.tile([P, P], MD, tag="at")
                        nc.vector.tensor_scalar_mul(at, tp, at_scale)
                        nc.tensor.matmul(ops[t], at, v_m,
                                         start=(m == 0), stop=(m == nm - 1))
                for t in range(nt):
                    osb = osb_pool.tile([P, D], F32, tag="osb")
                    nc.scalar.activation(osb, ops[t], AF.Copy,
                                         scale=1.0 / at_scale)
                    nc.sync.dma_start(out=out_c[P * t:P * (t + 1), :], in_=osb)

    nc.finalize()
    return nc


def _pack_T(X, cols):
    """[rows, D] fp32 -> transposed, chunk-packed [P, DC*cols] in MM_DT."""
    XT = np.ascontiguousarray(X.T)                     # [D, cols]
    out = np.empty((P, DC * cols), dtype=mybir.dt.np(MM_DT))
    for c in range(DC):
        out[:, c * cols:(c + 1) * cols] = XT[P * c:P * (c + 1), :]
    return out


def host_prep(inputs, nq=N // NCORES, nk=N, ncores=NCORES):
    """Build per-core input maps from the full problem inputs."""
    f = np.float32
    md = mybir.dt.np(MM_DT)
    c32 = lambda a: np.ascontiguousarray(np.asarray(a), dtype=f)
    Qu, Ku, Vu = c32(inputs["Q_user"]), c32(inputs["K_user"]), c32(inputs["V_user"])
    Qi, Ki = c32(inputs["Q_item"]), c32(inputs["K_item"])
    shared = {
        "kuT": _pack_T(Ku[:nk], nk),
        "kiT": _pack_T(Ki[:nk], nk),
        "vTp": _pack_T(Vu[:nk], nk),
        "Wqu": _pack_T(c32(inputs["W_q_user"]).T, D),
        "Wku": _pack_T(c32(inputs["W_k_user"]).T, D),
        "Wqi": _pack_T(c32(inputs["W_q_item"]).T, D),
        "Wki": _pack_T(c32(inputs["W_k_item"]).T, D),
        "Wv": _pack_T(c32(inputs["W_v_user"]).T, D),
        "bv": np.ascontiguousarray(c32(inputs["b_v_user"])[None, :], dtype=md),
        "bqu": c32(inputs["b_q_user"]).reshape(DC, P).T.copy(),
        "bku": c32(inputs["b_k_user"]).reshape(DC, P).T.copy(),
        "bqi": c32(inputs["b_q_item"]).reshape(DC, P).T.copy(),
        "bki": c32(inputs["b_k_item"]).reshape(DC, P).T.copy(),
        "ident": np.eye(P, dtype=f),
    }
    in_maps = []
    for c in range(ncores):
        m = dict(shared)
        m["quT"] = _pack_T(Qu[c * nq:(c + 1) * nq], nq)
        m["qiT"] = _pack_T(Qi[c * nq:(c + 1) * nq], nq)
        in_maps.append(m)
    return in_maps


_CACHED_NC = None


def run(inputs, trace=False, **kwargs):
    global _CACHED_NC
    if _CACHED_NC is None:
        _CACHED_NC = build_program()
    in_maps = host_prep(inputs)
    res = run_bass_kernel_spmd(_CACHED_NC, in_maps, core_ids=list(range(NCORES)),
                               trace=trace, **kwargs)
    out = np.concatenate([res.results[c]["out_c"] for c in range(NCORES)], axis=0)
    attn = np.concatenate([res.results[c]["attn_c"] for c in range(NCORES)], axis=0)
    return (out.astype(np.float32), attn.astype(np.float32)), res


def kernel(**inputs):
    (out, attn), _ = run(inputs)
    return out, attn
